# revision 3
# baseline (speedup 1.0000x reference)
"""Low-rank RNN (h' = 0.9h + 0.1*tanh(h) @ J^T + 0.1*u, J = m n^T rank-8)
on 8 Trainium2 NeuronCores, data-parallel over batch.

v2 design (per core: BS=8, T=512, H=1024=8x128, D=128, R=8):
  state h lives in PSUM slots [128 = h_lo, 64 = 8*h_hi + b], ring of 4 banks.
  per step chain:  ACT tanh (PSUM->SBUF bf16) -> PE proj (8 bf16 matmuls,
  nrep stationaries, N=8) -> DVE mask-broadcast (s replicated [64,8] ->
  block-diag [64,64] bf16 SBUF) -> PE expand (1 bf16 matmul, s2 stationary,
  accumulates onto hx-preseeded PSUM slot).
  off-chain: DVE hx = 0.9*h + u' written directly into the next PSUM slot;
  DVE copy h PSUM->SBUF ring; DMA ring->DRAM in p-major contiguous layout
  (host transposes back).
"""

import numpy as np
import ml_dtypes

B, T, D, H, R = 64, 512, 128, 1024, 8
NC = 8            # cores
BS = B // NC      # batch per core = 8
C = H // 128      # h chunks = 8
ALPHA = 0.1
DECAY = 1.0 - ALPHA
RING = 16

_CACHE = {}


def build(T_steps=T, debug=False):
    import concourse.mybir as mybir
    import concourse.tile as tile
    from concourse import bacc

    f32 = mybir.dt.float32
    bf16 = mybir.dt.bfloat16
    AF = mybir.ActivationFunctionType
    OP = mybir.AluOpType

    nc = bacc.Bacc("TRN2", target_bir_lowering=False, debug=debug)

    TB = T_steps * BS                       # columns of xt / u
    BLK = min(512, TB)
    NBLK = TB // BLK

    HC = C // 2   # half-chunk replication factor = 4

    xt_d = nc.dram_tensor("xt", [D, TB], bf16, kind="ExternalInput")
    itp_d = nc.dram_tensor("itp", [D, H], bf16, kind="ExternalInput")
    nrep_d = nc.dram_tensor("nrep", [128, C, HC * R], bf16, kind="ExternalInput")
    s2_d = nc.dram_tensor("s2", [C * R, 128], bf16, kind="ExternalInput")
    mask_d = nc.dram_tensor("mask", [HC * R, HC * BS], bf16, kind="ExternalInput")
    ident_d = nc.dram_tensor("ident", [128, 128], bf16, kind="ExternalInput")
    out_d = nc.dram_tensor("out", [128, T_steps, C * BS], f32, kind="ExternalOutput")

    def v3(ap):
        return ap.rearrange("p (c b) -> p c b", b=BS)

    with tile.TileContext(nc) as tc:
        with (
            tc.tile_pool(name="const", bufs=1) as constp,
            tc.tile_pool(name="upool", bufs=1) as upool,
            tc.tile_pool(name="xpool", bufs=1) as xpool,
            tc.tile_pool(name="th", bufs=4) as thp,
            tc.tile_pool(name="sbd", bufs=4) as sbdp,
            tc.tile_pool(name="ring", bufs=2) as ringp,
            tc.tile_pool(name="ps_h", bufs=4, space="PSUM") as ps_h,
            tc.tile_pool(name="ps_s", bufs=2, space="PSUM") as ps_s,
            tc.tile_pool(name="ps_u", bufs=2, space="PSUM") as ps_u,
        ):
            # ---- constants ----
            nrep_sb = constp.tile([128, C, HC * R], bf16)
            # s2 halves as separate tiles so both sit at partition base 0
            s2a_sb = constp.tile([HC * R, 128], bf16)
            s2b_sb = constp.tile([HC * R, 128], bf16)
            mask_sb = constp.tile([HC * R, HC * BS], bf16)
            ident_sb = constp.tile([128, 128], bf16)
            itp_sb = constp.tile([D, H], bf16)
            xt_sb = xpool.tile([D, TB], bf16)
            HR0 = HC * R
            nc.sync.dma_start(nrep_sb[:], nrep_d[:])
            nc.sync.dma_start(s2a_sb[:], s2_d[0:HR0, :])
            nc.sync.dma_start(s2b_sb[:], s2_d[HR0:2 * HR0, :])
            nc.sync.dma_start(mask_sb[:], mask_d[:])
            nc.sync.dma_start(ident_sb[:], ident_d[:])
            nc.sync.dma_start(itp_sb[:], itp_d[:])
            nc.sync.dma_start(xt_sb[:], xt_d[:])

            # ---- u' = 0.1 * x @ I^T staged into SBUF (bf16) ----
            # u_sb[p, c, t*BS+b] = 0.1 * u[b, t, 128c+p]
            u_sb = upool.tile([128, C, TB], bf16)

            def u_prep_pair(blk, c, nsplit=4):
                up = ps_u.tile([128, BLK], f32, name="up")
                # split the matmul into short quanta so the scheduler can
                # slot them into PE idle gaps without delaying the chain
                q = BLK // nsplit
                for k in range(nsplit):
                    nc.tensor.matmul(
                        up[:, k * q:(k + 1) * q],
                        itp_sb[:, c * 128:(c + 1) * 128],
                        xt_sb[:, blk * BLK + k * q:blk * BLK + (k + 1) * q],
                        start=True, stop=True,
                    )
                # alternate ACT / DVE for psum->sbuf copies
                dst = u_sb[:, c, blk * BLK:(blk + 1) * BLK]
                if c % 2 == 0:
                    nc.scalar.activation(dst, up[:], AF.Copy)
                else:
                    nc.vector.tensor_copy(dst, up[:])

            # block 0 upfront; blocks 1.. are interleaved into the recurrence
            for c in range(C):
                u_prep_pair(0, c)

            u_v = u_sb[:].rearrange("p c (t b) -> p c t b", b=BS)
            mask_v = mask_sb[:].rearrange("p (c b) -> p c b", b=BS)


            # ---- recurrence ----
            state = {"hr": None}

            def hcopy_and_flush(t):
                # copy h(t) PSUM -> SBUF ring, flush ring to DRAM when full
                g, s_ring = divmod(t, RING)
                if s_ring == 0:
                    state["hr"] = ringp.tile(
                        [128, RING, C * BS], f32, tag="hr", name="hr"
                    )
                hr = state["hr"]
                # ACT copy (Copy needs no ACT table swap; frees DVE)
                nc.scalar.activation(hr[:, s_ring, :], slots[t][:], AF.Copy)
                if s_ring == RING - 1 or t == T_steps - 1:
                    t0 = t - s_ring
                    nc.sync.dma_start(
                        out_d[:, t0:t0 + s_ring + 1, :], hr[:, :s_ring + 1, :]
                    )

            slots = {}
            for t in range(T_steps):
                slot = ps_h.tile([128, C * BS], f32, tag="h")
                slots[t] = slot
                hx_sb = thp.tile([128, C * BS], bf16, name="hx_sb",
                                 tag="hx_sb", bufs=3)
                if t == 0:
                    # h_0 = u'_0 staged in SBUF, seeded into PSUM by ident
                    nc.vector.tensor_copy(v3(hx_sb[:]), u_v[:, :, 0, :])
                    nc.tensor.matmul(slot[:], ident_sb[:], hx_sb[:],
                                     start=True, stop=True)
                else:
                    h_prev = slots[t - 1][:]
                    # chain 1 FIRST in issue order so the scheduler's sem
                    # chain lets tanh fire as soon as slot(t-1) completes.
                    # Split in halves: proj chunks 0-3 start while the second
                    # tanh half is still executing on ACT.
                    th = thp.tile([128, C * BS], bf16)
                    nc.scalar.activation(th[:], h_prev, AF.Tanh)
                    th_v = th[:].rearrange("p (c b) -> p c b", b=BS)
                    # off-chain: hx = 0.9*h_prev + u'_t staged in SBUF,
                    # then seeded into the PSUM slot by an identity matmul
                    # (start=True) so every bank group starts uniformly
                    nc.vector.scalar_tensor_tensor(
                        v3(hx_sb[:]), v3(h_prev), DECAY, u_v[:, :, t, :],
                        OP.mult, OP.add,
                    )
                    # delayed output copy of the PREVIOUS step (keeps tanh
                    # first in the consumer sem chain of each slot)
                    hcopy_and_flush(t - 1)
                    # chain 2: srep[(j,r), b] = sum_c n_c^T th_c, j-replicated
                    # over HC=4 chunks only (halves the LDWEIGHTS stream)
                    srep = ps_s.tile([HC * R, 1, BS], f32)
                    for c in range(C):
                        nc.tensor.matmul(
                            srep[:, 0, :],
                            nrep_sb[:, c, :],
                            th_v[:, c, :],
                            start=(c == 0), stop=(c == C - 1),
                        )
                    # chain 3: ONE half-size block-diag build [32, 32]; the
                    # same rhs serves both expand halves (content identical)
                    HR = HC * R
                    sbd = sbdp.tile([HR, HC, BS], bf16)
                    nc.vector.tensor_tensor(
                        sbd[:],
                        srep[:, 0:1, :].to_broadcast((HR, HC, BS)),
                        mask_v,
                        OP.mult,
                    )
                    sbd_f = sbd[:].rearrange("p c b -> p (c b)")
                    # seed slot = hx AFTER proj in PE queue order: executes
                    # during the mask window, off the critical chain
                    nc.tensor.matmul(slot[:], ident_sb[:], hx_sb[:],
                                     start=True, stop=False,
                                     skip_group_check=True)
                    # chain 4: two K=32 expands onto the preseeded slot,
                    # disjoint column halves; half 1 reads its stationary
                    # from s2 rows 32:64 into array rows 0:32
                    for half, s2h in enumerate((s2a_sb, s2b_sb)):
                        nc.tensor.matmul(
                            slot[:, half * HR:(half + 1) * HR],
                            s2h[:],
                            sbd_f,
                            start=False, stop=True,
                            skip_group_check=True,
                        )
                    # interleaved u-prep for blocks 1.. (one pair per step),
                    # issued AFTER the chain ops so its DVE/ACT copies queue
                    # behind mask/tanh instead of delaying them
                    if 8 <= t < 8 + (NBLK - 1) * C:
                        blk = 1 + (t - 8) // C
                        u_prep_pair(blk, (t - 8) % C)
            hcopy_and_flush(T_steps - 1)

    nc.compile()
    return nc


def prep_inputs(x, m, n, I, T_steps=T):
    """Host-side shard + layout prep (pure data marshaling)."""
    bf = ml_dtypes.bfloat16
    x = np.asarray(x, np.float32)
    m = np.asarray(m, np.float32)
    n = np.asarray(n, np.float32)
    I = np.asarray(I, np.float32)

    HC = C // 2
    itp = np.ascontiguousarray((ALPHA * I).T).astype(bf)         # [D, H]
    # nrep[p, c, j*R+r] = n[128c+p, r]  (replicated over HC=4 j's)
    nch = n.reshape(C, 128, R).transpose(1, 0, 2)                # [p, c, r]
    nrep = np.ascontiguousarray(np.tile(nch, (1, 1, HC))).astype(bf)
    # s2[j*R+r, p] = 0.1 * m[128j+p, r]
    mch = (ALPHA * m).reshape(C, 128, R)                         # [j, p, r]
    s2 = np.ascontiguousarray(mch.transpose(0, 2, 1).reshape(C * R, 128)).astype(bf)
    mask = np.kron(np.eye(HC, dtype=np.float32),
                   np.ones((R, BS), np.float32)).astype(bf)      # [32, 32]

    in_maps = []
    for core in range(NC):
        xs = x[core * BS:(core + 1) * BS, :T_steps]              # [BS, Ts, D]
        xt = np.ascontiguousarray(
            xs.transpose(2, 1, 0).reshape(D, T_steps * BS)).astype(bf)
        in_maps.append({
            "xt": xt, "itp": itp, "nrep": nrep, "s2": s2, "mask": mask,
            "ident": np.eye(128, dtype=ml_dtypes.bfloat16),
        })
    return in_maps


def kernel(x, m, n, I):
    from concourse.bass_utils import run_bass_kernel_spmd

    if "nc" not in _CACHE:
        _CACHE["nc"] = build()
    nc = _CACHE["nc"]

    in_maps = prep_inputs(x, m, n, I)
    res = run_bass_kernel_spmd(nc, in_maps, core_ids=list(range(NC)))
    outs = []
    for core in range(NC):
        arr = np.asarray(res.results[core]["out"], np.float32)   # [128, T, C*BS]
        # arr[p, t, c*BS+b] = h[b, t, c*128+p]
        full = arr.reshape(128, T, C, BS).transpose(3, 1, 2, 0).reshape(BS, T, H)
        outs.append(full)
    return np.concatenate(outs, axis=0)
